# revision 20
# baseline (speedup 1.0000x reference)
"""Bilinear sampling kernel for Trainium2 (Bass/Tile), data-parallel over 8 cores.

Problem: x[B=128, 224, 224, 5] f32; channels 0-2 an image, channel 3/4 sample
X/Y coordinates in [0, 223).  Output [B, 224, 224, 3]: bilinear sample of the
image at each (X, Y).  Coordinates are strictly interior, so fetches never
clip:
    out = w_tl*I[fy,fx] + w_tr*I[fy,fx+1] + w_bl*I[fy+1,fx] + w_br*I[fy+1,fx+1]
with fy = floor(Y), fx = floor(X).

Per NeuronCore (16 images, pure data parallel), pixels laid out on all 128
partitions (392 pixels per partition):
 1. Load raw interleaved rows; de-interleave channels on ACT; round-trip
    through DRAM to pair adjacent image rows per partition.
 2. Build a y-interleaved column table F[y,x] = [I[y,x,:], I[y+1,x,:]]
    (6 f32 per entry), then expand it into a 256B-slot table in DRAM:
       slot[k] = F entries [2k, 2k+2]   (18 f32 used of 64)
    so a pixel with entry e = fy*224+fx finds its whole 2x2 patch inside
    slot k = e>>1 at a 2-way position sel = e&1.
 3. Per-pixel gather of one 256B slot via the GPSIMD dma_gather ucode
    (InstDMAGatherAnt) spread over 4 SWDGE queues.  The wrapped index
    table (idx of stream position i at partition i%16, word i//16,
    replicated to all 8 GPSIMD core groups) is built without tiny-packet
    DMAs: k16 round-trips through DRAM with 784B runs into a [16, 8, 392]
    regroup, one strided ACT copy interleaves it to [16, 392, 8], and 7
    contiguous SBUF copies replicate it.  (The old per-element scatter
    produced ~800K 2-byte DMA packets per core and dominated runtime.)
 5. The per-image prep chain (weights, index table, slot table) for image
    b+1 is emitted before image b's gather/combine batch, so it executes
    under the gather phase instead of queueing behind it on the in-order
    engines; dtype-converting casts run on the idle ACT engine and the
    6-tap weighted sum is one DVE multiply plus one tensor_reduce.
 4. DVE folds the 2-way position select into the bilinear weights:
       out[c] = sum_{v in 0..2, ky in 0..1} W[v,ky] * slot[6v+3ky+c]
    with W[v,ky] = wy_ky * (s_sel*wx_v ...) via one-hot sel masks.
"""

import numpy as np

import concourse.bacc as bacc
import concourse.bass as bass
import concourse.mybir as mybir
from concourse.tile import TileContext

F32 = mybir.dt.float32
I32 = mybir.dt.int32
I16 = mybir.dt.int16
OP = mybir.AluOpType

H = W = 224
NIMG = 16          # images per core
NCORES = 8
NPART = 128        # pixel partitions
PIX = H * W // NPART   # 392 pixels per partition
FROWS = H - 1      # 223 F rows
SLOTS_PER_ROW = W // 2   # 112
NSLOT = FROWS * SLOTS_PER_ROW  # 24976 slots of 256B
NTAP = 6           # folded taps: v in 0..2, ky in 0..1
GCHUNKS = 14
CJ = PIX // GCHUNKS  # 28 pixels (j positions) per chunk


def build_module(n_imgs: int = NIMG) -> bass.Bass:
    nc = bacc.Bacc(num_swdge_queues=4)

    x_t = nc.dram_tensor("x", [n_imgs, H, W, 5], F32, kind="ExternalInput")
    out_t = nc.dram_tensor("out", [n_imgs, H, W, 3], F32, kind="ExternalOutput")

    x_flat = x_t[:].rearrange("n h w c -> n (h w c)")
    out_flat = out_t[:].rearrange("n h w c -> n (h w c)").rearrange(
        "n (p f) -> n p f", p=NPART
    )  # [n, 128, 1176]

    with TileContext(nc) as tc:
        with (
            tc.tile_pool(name="raw", bufs=2) as raw_pool,
            tc.tile_pool(name="mid", bufs=2) as mid_pool,
            tc.tile_pool(name="slotp", bufs=2) as slot_pool,
            tc.tile_pool(name="small", bufs=1) as small_pool,
            tc.tile_pool(name="gth", bufs=7) as gth_pool,
            tc.tile_pool(name="cmb", bufs=3) as cmb_pool,
            tc.tile_pool(name="dram", bufs=2, space="DRAM") as dram_pool,
        ):
            state = {}

            def prep(b):
                # ---- 1. raw load: partition p = pixels [392p, 392p+392)
                raw = raw_pool.tile([NPART, PIX * 5], F32)
                nc.sync.dma_start(
                    out=raw[:], in_=x_flat[b].rearrange("(p f) -> p f", p=NPART)
                )
                rawv = raw[:].rearrange("p (q c) -> p q c", c=5)
                x_ap = rawv[:, :, 3:4].rearrange("p q c -> p (q c)")
                y_ap = rawv[:, :, 4:5].rearrange("p q c -> p (q c)")

                # ---- 2. de-interleave image channels (ACT)
                img_rows = mid_pool.tile([NPART, PIX * 3], F32)
                nc.scalar.copy(
                    out=img_rows[:].rearrange("p (q c) -> p q c", c=3),
                    in_=rawv[:, :, 0:3],
                )
                imf = dram_pool.tile([H, W * 3], F32, name=f"imgflat{b}")
                nc.sync.dma_start(
                    out=imf[:].rearrange("h f -> (h f)").rearrange(
                        "(p f) -> p f", p=NPART
                    ),
                    in_=img_rows[:],
                )

                # ---- 3. slot table in DRAM, built per 128-row half
                slotd = dram_pool.tile([NSLOT, 64], F32, name=f"slots{b}")
                slotd_rows = slotd[:].rearrange("(r s) w -> r (s w)", s=SLOTS_PER_ROW)
                for h in range(2):
                    r0 = 128 * h
                    nrow = 128 if h == 0 else FROWS - 128  # 128 / 95
                    pp = mid_pool.tile([128, 1344], F32, tag="pp")
                    nc.sync.dma_start(out=pp[0:nrow, 0:672], in_=imf[r0:r0 + nrow])
                    nc.sync.dma_start(
                        out=pp[0:nrow, 672:1344], in_=imf[r0 + 1:r0 + nrow + 1]
                    )
                    # F row y: entry x = [I[y,x,:], I[y+1,x,:]]; 6 pad words at
                    # the end are garbage (only read into unused slot tails).
                    frow = mid_pool.tile([128, 1350], F32, tag="frow")
                    nc.scalar.copy(
                        out=frow[0:nrow, 0:1344].rearrange(
                            "p (xx k c) -> p xx k c", k=2, c=3
                        ),
                        in_=pp[0:nrow].rearrange("p (k xx c) -> p xx k c", k=2, c=3),
                    )
                    nc.vector.memset(frow[0:nrow, 1344:1350], 0)
                    # slot m of row y = F-row words [12m, 12m+18) — an
                    # overlapped-window AP (stride 12, width 18).
                    half_s = SLOTS_PER_ROW // 2
                    slotd_cols = slotd_rows[r0:r0 + nrow].rearrange(
                        "r (s w) -> r s w", w=64
                    )
                    # slot pad words (18..63) are never read by the combine,
                    # so slotbuf is left uninitialized.
                    for sc in range(2):
                        slotbuf = slot_pool.tile([128, half_s * 64], F32,
                                                 tag="slotbuf")
                        fr_ap = frow[0:nrow]
                        slot_src = bass.AP(
                            fr_ap.tensor,
                            fr_ap.offset + sc * half_s * 12,
                            [list(fr_ap.ap[0]), [12, half_s], [1, 18]],
                        )
                        nc.scalar.copy(
                            out=slotbuf[0:nrow].rearrange(
                                "p (s w) -> p s w", w=64
                            )[:, :, 0:18],
                            in_=slot_src,
                        )
                        nc.sync.dma_start(
                            out=slotd_cols[:, sc * half_s:(sc + 1) * half_s, :],
                            in_=slotbuf[0:nrow],
                        )

                # ---- 4. floors, fracs, weights, slot ids (DVE)
                def floor_of(src_ap, nm):
                    ri = small_pool.tile([NPART, PIX], I32, name=f"ri{nm}", tag="ri")
                    nc.scalar.copy(out=ri[:], in_=src_ap)
                    rf = small_pool.tile([NPART, PIX], F32, name=f"rf{nm}", tag="rf")
                    nc.scalar.copy(out=rf[:], in_=ri[:])
                    gt = small_pool.tile([NPART, PIX], F32, name=f"gt{nm}", tag="gt")
                    nc.vector.tensor_tensor(
                        out=gt[:], in0=rf[:], in1=src_ap, op=OP.is_gt
                    )
                    fl = small_pool.tile([NPART, PIX], F32, name=f"fl{nm}",
                                         tag=f"fl{nm}")
                    nc.vector.tensor_tensor(
                        out=fl[:], in0=rf[:], in1=gt[:], op=OP.subtract
                    )
                    return fl

                fxf = floor_of(x_ap, "x")
                fyf = floor_of(y_ap, "y")
                wx1 = small_pool.tile([NPART, PIX], F32)
                nc.vector.tensor_tensor(out=wx1[:], in0=x_ap, in1=fxf[:],
                                        op=OP.subtract)
                wy1 = small_pool.tile([NPART, PIX], F32)
                nc.vector.tensor_tensor(out=wy1[:], in0=y_ap, in1=fyf[:],
                                        op=OP.subtract)
                wx0 = small_pool.tile([NPART, PIX], F32)
                nc.vector.tensor_scalar(out=wx0[:], in0=wx1[:], scalar1=-1.0,
                                        scalar2=1.0, op0=OP.mult, op1=OP.add)
                wy0 = small_pool.tile([NPART, PIX], F32)
                nc.vector.tensor_scalar(out=wy0[:], in0=wy1[:], scalar1=-1.0,
                                        scalar2=1.0, op0=OP.mult, op1=OP.add)

                ef = small_pool.tile([NPART, PIX], F32)
                nc.vector.scalar_tensor_tensor(
                    out=ef[:], in0=fyf[:], scalar=float(W), in1=fxf[:],
                    op0=OP.mult, op1=OP.add,
                )
                ei = small_pool.tile([NPART, PIX], I32)
                nc.scalar.copy(out=ei[:], in_=ef[:])
                ki = small_pool.tile([NPART, PIX], I32)
                nc.vector.tensor_scalar(out=ki[:], in0=ei[:], scalar1=1,
                                        scalar2=None, op0=OP.arith_shift_right)
                k16 = small_pool.tile([NPART, PIX], I16)
                nc.scalar.copy(out=k16[:], in_=ki[:])
                seli = small_pool.tile([NPART, PIX], I32)
                nc.vector.tensor_scalar(out=seli[:], in0=ei[:], scalar1=1,
                                        scalar2=None, op0=OP.bitwise_and)
                self_f = small_pool.tile([NPART, PIX], F32)
                nc.scalar.copy(out=self_f[:], in_=seli[:])
                # one-hot masks s_t = (sel == t), t in 0..1
                s_m = []
                for t in range(2):
                    st = small_pool.tile([NPART, PIX], F32, name=f"s{t}",
                                         tag=f"s{t}")
                    nc.vector.tensor_scalar(out=st[:], in0=self_f[:],
                                            scalar1=float(t), scalar2=None,
                                            op0=OP.is_equal)
                    s_m.append(st)
                # U_v = s_v*wx0 + s_{v-1}*wx1  (s out of range = 0)
                us = []
                for v in range(3):
                    uv = small_pool.tile([NPART, PIX], F32, name=f"u{v}",
                                         tag=f"u{v}")
                    if v == 0:
                        nc.vector.tensor_tensor(out=uv[:], in0=s_m[0][:],
                                                in1=wx0[:], op=OP.mult)
                    elif v == 2:
                        nc.vector.tensor_tensor(out=uv[:], in0=s_m[1][:],
                                                in1=wx1[:], op=OP.mult)
                    else:
                        ua = small_pool.tile([NPART, PIX], F32, tag="ua")
                        nc.vector.tensor_tensor(out=ua[:], in0=s_m[v][:],
                                                in1=wx0[:], op=OP.mult)
                        ub = small_pool.tile([NPART, PIX], F32, tag="ub")
                        nc.vector.tensor_tensor(out=ub[:], in0=s_m[v - 1][:],
                                                in1=wx1[:], op=OP.mult)
                        nc.vector.tensor_tensor(out=uv[:], in0=ua[:], in1=ub[:],
                                                op=OP.add)
                    us.append(uv)
                # wcat[p, t*PIX + j] = weight of tap t = 2v+ky for pixel j
                # (matches slot word layout w = 3t + c).
                wcat = small_pool.tile([NPART, NTAP * PIX], F32, tag="wcat",
                                       bufs=2)
                for v, uv in enumerate(us):
                    for ky, wyk in enumerate((wy0, wy1)):
                        t = 2 * v + ky
                        nc.vector.tensor_tensor(
                            out=wcat[:, t * PIX:(t + 1) * PIX],
                            in0=uv[:], in1=wyk[:], op=OP.mult,
                        )

                # ---- 5. wrapped + replicated idx table, without tiny packets:
                # k16 -> DRAM (contiguous); load [16, 8, 392] regroup with 784B
                # runs; DVE-interleave to [16, 392, 8]; replicate to 128.
                kd = dram_pool.tile([NPART, PIX], I16, name=f"k16d{b}")
                nc.scalar.dma_start(out=kd[:], in_=k16[:])
                kt = small_pool.tile([16, 8 * PIX], I16, tag="kt", bufs=2)
                kd_ap = kd[:]
                kt_src = bass.AP(
                    kd_ap.tensor,
                    kd_ap.offset,
                    [[PIX, 16], [16 * PIX, 8], [1, PIX]],
                )
                nc.scalar.dma_start(
                    out=kt[:].rearrange("q (a j) -> q a j", j=PIX),
                    in_=kt_src,
                )
                widx = small_pool.tile([128, PIX * 8], I16, tag="widx", bufs=2)
                nc.scalar.copy(
                    out=widx[0:16, :].rearrange("q (j a) -> q j a", a=8),
                    in_=kt[:].rearrange("q (a j) -> q j a", j=PIX),
                )
                for g in range(1, 8):
                    nc.scalar.dma_start(
                        out=widx[16 * g:16 * (g + 1), :], in_=widx[0:16, :]
                    )

                state[b] = (slotd, widx, wcat)

            def gather_combine(b):
                slotd, widx, wcat = state.pop(b)
                # ---- 6. gather + combine + store, 14 chunks on 4 queues
                for ck in range(GCHUNKS):
                    gth = gth_pool.tile([128, CJ * 64], F32, tag="gth")
                    nc.gpsimd.dma_gather(
                        out_ap=gth[:].rearrange("p (n w) -> p n w", w=64),
                        in_ap=slotd[:],
                        idxs_ap=widx[:, CJ * 8 * ck: CJ * 8 * (ck + 1)],
                        num_idxs=CJ * 128,
                        num_idxs_reg=CJ * 128,
                        elem_size=64,
                        single_packet=False,
                        queue_num=ck % 4,
                    )
                    # weighted taps: tmp[p, q, c, t] = G[p, q, 3t+c] * W6[t][p, q]
                    # then pool_avg over the 6-tap window = the bilinear sum.
                    gb = gth[:]
                    g_ap = bass.AP(
                        gb.tensor,
                        gb.offset,
                        [list(gb.ap[0]), [64, CJ], [1, 3], [3, NTAP]],
                    )
                    wc = wcat[:]
                    w_ap = bass.AP(
                        wc.tensor,
                        wc.offset + CJ * ck,
                        [list(wc.ap[0]), [1, CJ], [0, 3], [PIX, NTAP]],
                    )
                    tmp = cmb_pool.tile([NPART, CJ * 3 * NTAP], F32, tag="tmp")
                    nc.vector.tensor_tensor(
                        out=tmp[:].rearrange("p (q c t) -> p q c t", c=3, t=NTAP),
                        in0=g_ap, in1=w_ap, op=OP.mult,
                    )
                    acc = cmb_pool.tile([NPART, CJ * 3], F32, tag="acc")
                    nc.vector.tensor_reduce(
                        out=acc[:],
                        in_=tmp[:].rearrange("p (q t) -> p q t", t=NTAP),
                        axis=mybir.AxisListType.X,
                        op=OP.add,
                    )
                    nc.sync.dma_start(
                        out=out_flat[b][:, CJ * 3 * ck: CJ * 3 * (ck + 1)],
                        in_=acc[:],
                    )

            # software pipeline: prep(b+1) is emitted before gather_combine(b)
            # so the next image's DVE/ACT/DMA index+table chain runs under the
            # current image's gather phase instead of queueing behind it.
            prep(0)
            for b in range(n_imgs):
                if b + 1 < n_imgs:
                    prep(b + 1)
                gather_combine(b)

    nc.compile()
    return nc


def kernel(x: np.ndarray) -> np.ndarray:
    """Full-input entry point: shards the batch over 8 NeuronCores."""
    from concourse import bass_utils

    B = x.shape[0]
    assert x.shape == (B, H, W, 5) and B % NCORES == 0
    per = B // NCORES
    nc = build_module(per)
    in_maps = [
        {"x": np.ascontiguousarray(x[c * per:(c + 1) * per])} for c in range(NCORES)
    ]
    res = bass_utils.run_bass_kernel_spmd(nc, in_maps, core_ids=list(range(NCORES)))
    out = np.concatenate([res.results[c]["out"] for c in range(NCORES)], axis=0)
    return out
